# revision 4
# baseline (speedup 1.0000x reference)
"""Trainium2 Bass kernel: GNN message passing (iterative Laplacian diffusion).

Problem: u0 = F@Ws + bs + elu(F@W1 + b1)@W2 + b2;  16x: u <- u - 0.1*(L@u)
  F: [16384, 512] fp32, L: [16384, 16384] fp32, output u: [16384, 16] fp32.

Strategy (8 NeuronCores, row-parallel SpMM):
  - Shard L row-wise (2048 rows/core).  The 16 diffusion steps re-read the
    whole L shard each step => memory-bound on HBM.  We compress L to
    fp8e4 (scaled by sigma2 * 2^17 so entries sit in e4m3's sweet spot),
    halving-again traffic vs bf16: 32 MiB/step/core.  The carried state u
    stays fp32; only the matmul operands are fp8, and each step's update is
    ~1e-3 of u, so quantization error lands at ~1e-4 relative overall.
  - Host pre-transposes the shard (lhsT layout) and pre-permutes its
    128-row k-chunks so chunk j of core i is logical chunk (16*i+j) % 128:
    every core starts each step with its OWN 16 chunks, whose u-tiles come
    from the local AXPY (no collective wait), hiding the per-step AllGather
    (~5us) behind the first 16/128 of the matmul work.  The other 112
    chunks read u from a doubled (wrap-free) SBUF buffer at a per-core
    register offset (rot input) — same SPMD instruction stream on all cores.
  - PE mapping: lhsT = L^T block [128,128] fp8 (FWL 4B/cycle weight load),
    rhs = u chunk [128,16] fp8, accumulate 128 k-chunks into one PSUM bank
    holding all 16 row-tiles as 16-column slices ([128, 256] fp32).
"""

import numpy as np
import ml_dtypes
from dataclasses import dataclass

from concourse import bass, bacc, tile
import concourse.mybir as mybir
from concourse.bass_utils import run_bass_kernel_spmd

F32 = mybir.dt.float32
FP8 = mybir.dt.float8e4
U32 = mybir.dt.uint32
P = 128  # partitions


@dataclass(frozen=True)
class Cfg:
    C: int = 8          # cores
    N: int = 16384      # nodes
    IN_F: int = 512     # input features
    HID: int = 256      # hidden dim
    OUT: int = 16       # output features
    STEPS: int = 16
    SIGMA2: float = 0.1
    SCALE: float = 2.0 ** 17   # fp8 encoding scale for (sigma2*L)

    @property
    def R(self):   # rows per core
        return self.N // self.C

    @property
    def MT(self):  # row-tiles (= own k-chunks) per core
        return self.R // P

    @property
    def KC(self):  # total k-chunks
        return self.N // P

    @property
    def NTILE(self):  # phase-1 rhs tile width
        return min(512, self.R)


def build_program(cfg: Cfg):
    C, R, MT, KC, OUT, STEPS = cfg.C, cfg.R, cfg.MT, cfg.KC, cfg.OUT, cfg.STEPS
    IN_F, HID = cfg.IN_F, cfg.HID
    KI = IN_F // P   # 4 input-feature k-chunks
    KH = HID // P    # 2 hidden k-chunks
    NT = cfg.NTILE
    AXPY_C = -1.0 / cfg.SCALE

    nc = bacc.Bacc("TRN2", target_bir_lowering=False, debug=False,
                   enable_asserts=False, num_devices=C)

    lapT = nc.dram_tensor("lapT", [cfg.N, R], FP8, kind="ExternalInput")
    featT = nc.dram_tensor("featT", [P, KI * R], F32, kind="ExternalInput")
    w1_t = nc.dram_tensor("w1_t", [P, KI * KH * P], F32, kind="ExternalInput")
    ws_t = nc.dram_tensor("ws_t", [P, KI * OUT], F32, kind="ExternalInput")
    w2_t = nc.dram_tensor("w2_t", [P, KH * OUT], F32, kind="ExternalInput")
    b1_t = nc.dram_tensor("b1_t", [P, KH], F32, kind="ExternalInput")
    biasO = nc.dram_tensor("biasO", [P, MT * OUT], F32, kind="ExternalInput")
    rot = nc.dram_tensor("rot", [1, 1], U32, kind="ExternalInput")
    out_u = nc.dram_tensor("out_u", [R, OUT], F32, kind="ExternalOutput")

    AF = mybir.ActivationFunctionType
    ALU = mybir.AluOpType

    with tile.TileContext(nc) as tc:
        with (
            tc.tile_pool(name="slabp", bufs=6) as slabp,
            tc.tile_pool(name="upool", bufs=2) as upool,
            tc.tile_pool(name="ownp", bufs=2) as ownp,
            tc.tile_pool(name="u32p", bufs=2) as u32p,
            tc.tile_pool(name="constp", bufs=1) as constp,
            tc.tile_pool(name="zpsum", bufs=2, space="PSUM") as zpsum,
            tc.tile_pool(name="dramp", bufs=2, space="DRAM") as dramp,
        ):
            # per-core rotation offset -> PE register (for dynamic rhs APs)
            rot_s = constp.tile([1, 1], U32, name="rot_s")
            nc.sync.dma_start(rot_s[:], rot[:])
            # zero weights: one matmul writes a whole PSUM bank of zeros with
            # start=True (clears has_written bank-wide); the 2048 real matmuls
            # then run start=False and accumulate per-element.
            zt = constp.tile([P, max(2 * P, MT * OUT)], FP8, name="zt")
            nc.vector.memset(zt[:], 0)
            reg = nc.alloc_registers("rotreg", engines=[mybir.EngineType.PE])
            nc.regs_load(reg, rot_s[0:1, 0:1])
            sv_base = nc.snap(reg, donate=True, min_val=0,
                              max_val=(C - 1) * MT * OUT)

            u32_cur = u32p.tile([P, MT * OUT], F32, name="u32_init", tag="u32")

            # ---------------- MLP head: u0 = F@Ws + bs + elu(F@W1+b1)@W2 + b2
            with (
                tc.tile_pool(name="mlpp", bufs=1) as mlpp,
                tc.tile_pool(name="tmpp", bufs=2) as tmpp,
                tc.tile_pool(name="mps1", bufs=2, space="PSUM") as mps1,
                tc.tile_pool(name="mps2", bufs=2, space="PSUM") as mps2,
            ):
                featT_s = mlpp.tile([P, KI * R], F32, name="featT_s")
                nc.sync.dma_start(featT_s[:], featT[:])
                w1_s = mlpp.tile([P, KI * KH * P], F32, name="w1_s")
                nc.sync.dma_start(w1_s[:], w1_t[:])
                ws_s = mlpp.tile([P, KI * OUT], F32, name="ws_s")
                nc.sync.dma_start(ws_s[:], ws_t[:])
                w2_s = mlpp.tile([P, KH * OUT], F32, name="w2_s")
                nc.sync.dma_start(w2_s[:], w2_t[:])
                b1_s = mlpp.tile([P, KH], F32, name="b1_s")
                nc.sync.dma_start(b1_s[:], b1_t[:])
                biasO_s = mlpp.tile([P, MT * OUT], F32, name="biasO_s")
                nc.sync.dma_start(biasO_s[:], biasO[:])

                # phase 1: hT[kappa, mt*R + n] = elu(F@W1 + b1)^T
                hT = mlpp.tile([P, KH * R], F32, name="hT")
                for mt in range(KH):
                    for nt in range(R // NT):
                        ps = mps1.tile([P, NT], F32, name="ps1", tag="ps1")
                        for k in range(KI):
                            nc.tensor.matmul(
                                ps[:],
                                w1_s[:, (k * KH + mt) * P:(k * KH + mt + 1) * P],
                                featT_s[:, k * R + nt * NT: k * R + (nt + 1) * NT],
                                start=(k == 0), stop=(k == KI - 1),
                            )
                        b1_ap = b1_s[:, mt:mt + 1]
                        t_relu = tmpp.tile([P, NT], F32, name="t_relu", tag="t_relu")
                        nc.scalar.activation(t_relu[:], ps[:], AF.Relu, bias=b1_ap)
                        t_min = tmpp.tile([P, NT], F32, name="t_min", tag="t_min")
                        nc.vector.tensor_scalar(t_min[:], ps[:], b1_ap, 0.0,
                                                ALU.add, ALU.min)
                        t_exp = tmpp.tile([P, NT], F32, name="t_exp", tag="t_exp")
                        nc.scalar.activation(t_exp[:], t_min[:], AF.Exp)
                        nc.vector.scalar_tensor_tensor(
                            hT[:, mt * R + nt * NT: mt * R + (nt + 1) * NT],
                            t_exp[:], -1.0, t_relu[:], ALU.add, ALU.add)

                # phase 2: u0 row-tiles
                for rt in range(MT):
                    ps2 = mps2.tile([P, OUT], F32, name="ps2", tag="ps2")
                    for k in range(KI):
                        nc.tensor.matmul(
                            ps2[:],
                            featT_s[:, k * R + rt * P: k * R + rt * P + P],
                            ws_s[:, k * OUT:(k + 1) * OUT],
                            start=(k == 0), stop=False,
                        )
                    for k2 in range(KH):
                        nc.tensor.matmul(
                            ps2[:],
                            hT[:, k2 * R + rt * P: k2 * R + rt * P + P],
                            w2_s[:, k2 * OUT:(k2 + 1) * OUT],
                            start=False, stop=(k2 == KH - 1),
                        )
                    nc.vector.tensor_tensor(
                        u32_cur[:, rt * OUT:(rt + 1) * OUT], ps2[:],
                        biasO_s[:, rt * OUT:(rt + 1) * OUT], ALU.add)

            # ---------------- diffusion loop
            def cast_and_gather(u32_tile):
                """fp32 u shard -> fp8 own_buf; AllGather into doubled u_buf2."""
                own = ownp.tile([P, MT * OUT], FP8, name="own", tag="own")
                nc.scalar.activation(own[:], u32_tile[:], AF.Copy)
                agin = dramp.tile([P, MT * OUT], FP8, name="agin", tag="agin")
                agout = dramp.tile([C * P, MT * OUT], FP8, name="agout",
                                   tag="agout", addr_space="Shared")
                nc.gpsimd.dma_start(agin[:], own[:])
                nc.gpsimd.collective_compute(
                    "AllGather", ALU.bypass,
                    replica_groups=[list(range(C))],
                    ins=[agin.opt()], outs=[agout.opt()],
                )
                ub = upool.tile([P, 2 * KC * OUT], FP8, name="ub", tag="ub")
                src = agout[:].rearrange("(r k) m -> k r m", k=P)
                for h in range(2):
                    dst = ub[:, h * KC * OUT:(h + 1) * KC * OUT]
                    nc.gpsimd.dma_start(
                        dst.rearrange("k (r m) -> k r m", r=C), src)
                return own, ub

            own_cur, ub_cur = cast_and_gather(u32_cur)

            for t in range(STEPS):
                zp = zpsum.tile([P, MT * OUT], F32, name="zp", tag="zp")
                nc.tensor.matmul(zp[:], zt[:, 0:P], zt[:, 0:MT * OUT],
                                 start=True, stop=False)
                ubw = ub_cur[:, bass.ds(sv_base, KC * OUT)]
                for j in range(KC):
                    slab = slabp.tile([P, R], FP8, name="slab", tag="slab")
                    nc.sync.dma_start(slab[:], lapT[j * P:(j + 1) * P, :])
                    if j < MT:
                        rhs = own_cur[:, j * OUT:(j + 1) * OUT]
                    else:
                        rhs = ubw[:, j * OUT:(j + 1) * OUT]
                    for m in range(MT):
                        nc.tensor.matmul(
                            zp[:, m * OUT:(m + 1) * OUT],
                            slab[:, m * P:(m + 1) * P],
                            rhs,
                            start=False,
                            stop=(j == KC - 1 and m == MT - 1),
                        )
                u32_new = u32p.tile([P, MT * OUT], F32, name="u32", tag="u32")
                nc.vector.scalar_tensor_tensor(
                    u32_new[:], zp[:], AXPY_C, u32_cur[:], ALU.mult, ALU.add)
                u32_cur = u32_new
                if t < STEPS - 1:
                    own_cur, ub_cur = cast_and_gather(u32_cur)
                else:
                    nc.gpsimd.dma_start(
                        out_u[:].rearrange("(m k) j -> k m j", k=P),
                        u32_cur[:].rearrange("k (m j) -> k m j", j=OUT))

    nc.compile()
    return nc


def host_prep(cfg: Cfg, features, laplacian, W1, b1, W2, b2, Ws, bs):
    C, R, MT, KC, OUT = cfg.C, cfg.R, cfg.MT, cfg.KC, cfg.OUT
    KI, KH = cfg.IN_F // P, cfg.HID // P
    F = np.ascontiguousarray(np.asarray(features, np.float32))
    L = np.asarray(laplacian, np.float32)
    W1 = np.asarray(W1, np.float32)
    b1 = np.asarray(b1, np.float32)
    W2 = np.asarray(W2, np.float32)
    b2 = np.asarray(b2, np.float32)
    Ws = np.asarray(Ws, np.float32)
    bs = np.asarray(bs, np.float32)

    Lq = (L * np.float32(cfg.SIGMA2 * cfg.SCALE)).astype(ml_dtypes.float8_e4m3)

    w1_t = np.ascontiguousarray(
        W1.reshape(KI, P, KH, P).transpose(1, 0, 2, 3).reshape(P, KI * KH * P))
    ws_t = np.ascontiguousarray(
        Ws.reshape(KI, P, OUT).transpose(1, 0, 2).reshape(P, KI * OUT))
    w2_t = np.ascontiguousarray(
        W2.reshape(KH, P, OUT).transpose(1, 0, 2).reshape(P, KH * OUT))
    b1_t = np.ascontiguousarray(b1.reshape(KH, P).T)
    biasO = np.ascontiguousarray(np.tile((bs + b2).astype(np.float32), (P, MT)))

    in_maps = []
    for i in range(C):
        shard = Lq[i * R:(i + 1) * R, :]                   # [R, N]
        Ti = np.ascontiguousarray(shard.T)                 # [N, R] lhsT layout
        perm = [(MT * i + j) % KC for j in range(KC)]
        Ti_p = np.ascontiguousarray(
            Ti.reshape(KC, P, R)[perm].reshape(cfg.N, R))
        Fi = F[i * R:(i + 1) * R, :]
        featT_i = np.ascontiguousarray(
            Fi.T.reshape(KI, P, R).transpose(1, 0, 2).reshape(P, KI * R))
        in_maps.append({
            "lapT": Ti_p,
            "featT": featT_i,
            "w1_t": w1_t,
            "ws_t": ws_t,
            "w2_t": w2_t,
            "b1_t": b1_t,
            "biasO": biasO,
            "rot": np.array([[i * MT * OUT]], np.uint32),
        })
    return in_maps


_NC_CACHE = {}


def _get_nc(cfg: Cfg):
    if cfg not in _NC_CACHE:
        _NC_CACHE[cfg] = build_program(cfg)
    return _NC_CACHE[cfg]


def _install_ntff_hook():
    """Recreate antenv.axon_hooks (absent in this image) so
    run_bass_kernel_spmd(trace=True) can NTFF-profile via libaxon_pjrt."""
    import sys
    import types
    import ctypes
    import contextlib

    if "antenv.axon_hooks" in sys.modules:
        return
    so_path = "/opt/axon/libaxon_pjrt.so"
    lib = ctypes.CDLL(so_path)
    if not hasattr(lib, "axon_start_nrt_profile"):
        return
    lib.axon_start_nrt_profile.argtypes = [
        ctypes.POINTER(ctypes.c_int64), ctypes.c_size_t]
    lib.axon_start_nrt_profile.restype = ctypes.c_int64
    lib.axon_stop_nrt_profile.argtypes = [ctypes.c_char_p]
    lib.axon_stop_nrt_profile.restype = ctypes.c_int64

    @contextlib.contextmanager
    def _hook(output_dir, device_ids):
        import jax
        jax.devices()
        if device_ids:
            ids = (ctypes.c_int64 * len(device_ids))(*device_ids)
            rc = lib.axon_start_nrt_profile(ids, len(device_ids))
        else:
            rc = lib.axon_start_nrt_profile(None, 0)
        if rc != 0:
            raise RuntimeError(f"axon_start_nrt_profile rc={rc}")
        try:
            yield
        finally:
            n = lib.axon_stop_nrt_profile(str(output_dir).encode())
            print(f"profile: {n} file(s) written to {output_dir}")

    mod = types.ModuleType("antenv.axon_hooks")
    mod.get_axon_ntff_profile_hook = lambda: _hook
    mod.set_axon_ntff_profile_hook = lambda h: None
    sys.modules["antenv.axon_hooks"] = mod


def run(inputs, cfg: Cfg = Cfg(), trace: bool = False):
    if trace:
        _install_ntff_hook()
    nc = _get_nc(cfg)
    in_maps = host_prep(cfg, **inputs)
    res = run_bass_kernel_spmd(nc, in_maps, core_ids=list(range(cfg.C)),
                               trace=trace)
    out = np.concatenate([res.results[i]["out_u"] for i in range(cfg.C)], axis=0)
    return out, res


def kernel(**inputs):
    out, _ = run(inputs)
    return out
